# revision 26
# baseline (speedup 1.0000x reference)
"""WaveNet-like dense CNN on 8 TRN2 NeuronCores — batch data parallel.

v4: factorized residual conv. Block i's conv-psum is computed as
  taps_i(h_{i-1}) + (wd@wo)-taps_i(fg_{i-1}) + bias_hi_i [+ boundary delta]
so the cross-block critical chain is fg_{i-1} -> tapsfg -> relu -> gate
mms -> tanh/sig -> fg_i, skipping the wo-matmul and h-update stages.
h itself updates off-chain with a full block of slack. bias_hi folds
bd_i + (wd1_i+wd0_i)@bo_{i-1} into the relu bias; a tiny K=1 matmul
fixes the zero-pad boundary columns on tc=0 partition groups.
"""

import sys

sys.path.insert(0, "/opt/trn_rl_repo")

import numpy as np
import ml_dtypes

import concourse.bass as bass
import concourse.tile as tile
from concourse import mybir
from concourse.bass_utils import run_bass_kernel_spmd

F32 = mybir.dt.float32
F32R = mybir.dt.float32r
BF16 = mybir.dt.bfloat16
AF = mybir.ActivationFunctionType
ALU = mybir.AluOpType

SEQ_LEN = 2048
C = 8
SK = 256
NB = 32
B = 32
N_CORES = 8
BL = B // N_CORES      # 4 samples per core
TCH = 4                # time chunks per sample
L = SEQ_LEN // TCH     # 512 free elems
NW = 9                 # packed weight strips per block


def _dil(i):
    return 2 ** (i % 8)


# ---------------------------------------------------------------- build


def _build_nc(fixup=True):
    nc = bass.Bass("TRN2", target_bir_lowering=False, debug=False,
                   num_devices=N_CORES)

    def din(name, shape, dt=F32R):
        return nc.dram_tensor(name, shape, dt, kind="ExternalInput").ap()

    x_d = din("x", [BL * TCH, L])
    win_d = din("win", [BL * TCH, 128])
    # per-block strips packed contiguously (order: wd1, wd0, wd0s, wf, wg,
    # wo, wd1o, wd0o, wd0so); one DMA per block
    wall_d = din("wall", [NB, 128, NW * 128], BF16)
    wdelta_d = din("wdelta", [1, NB * 128], BF16)
    ones_d = din("ones", [1, 128], BF16)
    bin_d = din("bin", [128, 1], F32)
    bh_d = din("bh", [128, NB], F32)
    bf_d = din("bf", [128, NB], F32)
    bg_d = din("bg", [128, NB], F32)
    bo_d = din("bo", [128, NB], F32)
    wsk_d = din("wsk", [32, 16 * 128], BF16)
    ident_d = din("ident", [128, 128])
    bsk_d = din("bsk", [128, 2], F32)
    wo1_d = din("wo1", [128, 4 * 128], BF16)
    bo1_d = din("bo1", [128, 2], F32)
    wo2_d = din("wo2", [128, 4 * 128], BF16)
    bo2_d = din("bo2", [128, 2], F32)

    out_d = nc.dram_tensor("out", [BL, SK], F32, kind="ExternalOutput").ap()

    with tile.TileContext(nc) as tc:
        _emit(nc, tc, x_d, win_d, wall_d, wdelta_d, ones_d,
              bin_d, bh_d, bf_d, bg_d, bo_d, wsk_d, ident_d, bsk_d,
              wo1_d, bo1_d, wo2_d, bo2_d, out_d)

    if fixup:
        _split_excess_waits(nc)
    return nc


def _emit(nc, tc, x_d, win_d, wall_d, wdelta_d, ones_d,
          bin_d, bh_d, bf_d, bg_d, bo_d, wsk_d, ident_d, bsk_d,
          wo1_d, bo1_d, wo2_d, bo2_d, out_d):
    from contextlib import ExitStack
    ctx = ExitStack()
    const = ctx.enter_context(tc.tile_pool(name="const", bufs=1))
    work = ctx.enter_context(tc.tile_pool(name="work", bufs=2))
    hpool = ctx.enter_context(tc.tile_pool(name="h", bufs=3))
    pspool = ctx.enter_context(tc.tile_pool(name="ps", bufs=3, space="PSUM"))
    pspool_f = ctx.enter_context(tc.tile_pool(name="psf", bufs=1, space="PSUM"))
    pspool_g = ctx.enter_context(tc.tile_pool(name="psg", bufs=1, space="PSUM"))
    pspool_o = ctx.enter_context(tc.tile_pool(name="pso", bufs=2, space="PSUM"))
    pspool1 = ctx.enter_context(tc.tile_pool(name="ps1", bufs=1, space="PSUM"))

    # warm the Act engine's activation table (sigmoid_and_others covers
    # identity/relu/tanh/sigmoid) so the 1.3us table load overlaps the
    # initial DMA wait instead of blocking the first real activation
    warm = const.tile([1, 2], F32, tag="warm")
    nc.vector.memset(warm[:], 0.0)
    nc.scalar.activation(warm[:], warm[:], AF.Sigmoid)

    # ---- persistent loads (x/win/bias + first blocks' weights first so
    # compute starts early; the rest streams behind)
    x_sb = const.tile([BL * TCH, L], F32R, tag="x")
    nc.sync.dma_start(x_sb[:], x_d[:])
    win_sb = const.tile([BL * TCH, 128], F32R, tag="win")
    nc.sync.dma_start(win_sb[:], win_d[:])

    btiles = {}
    wall_sb = const.tile([128, NB * NW * 128], BF16, tag="wall")

    def dma_wall(i):
        nc.sync.dma_start(
            wall_sb[:, NW * 128 * i:NW * 128 * (i + 1)], wall_d[i])

    bspecs = (("bin", bin_d, 1), ("bh", bh_d, NB), ("bf", bf_d, NB),
              ("bg", bg_d, NB), ("bo", bo_d, NB), ("bsk", bsk_d, 2),
              ("bo1", bo1_d, 2), ("bo2", bo2_d, 2))
    for nm, d, w in bspecs:
        btiles[nm] = const.tile([128, w], F32, tag=nm, name=f"b_{nm}")
    # issue the startup-critical transfers on idle engines' queues so they
    # run in parallel instead of serializing on the sync queue
    nc.scalar.dma_start(btiles["bin"][:], bin_d[:])
    nc.gpsimd.dma_start(
        wall_sb[:, 0:NW * 128], wall_d[0])
    nc.scalar.dma_start(btiles["bh"][:], bh_d[:])
    wdelta_sb = const.tile([1, NB * 128], BF16, tag="wdelta")
    nc.sync.dma_start(wdelta_sb[:], wdelta_d[:])
    ones_sb = const.tile([1, 128], BF16, tag="ones")
    nc.sync.dma_start(ones_sb[:], ones_d[:])
    nc.gpsimd.dma_start(
        wall_sb[:, NW * 128:2 * NW * 128], wall_d[1])
    for nm, d, w in bspecs:
        if nm not in ("bin", "bh"):
            nc.sync.dma_start(btiles[nm][:], d[:])
    for i in range(2, NB):
        dma_wall(i)

    def wstrip(i, j):
        o = NW * 128 * i + 128 * j
        return wall_sb[:, o:o + 128]

    # tail weights last (only needed at the very end)
    wsk_sb = const.tile([32, 16 * 128], BF16, tag="wsk")
    nc.sync.dma_start(wsk_sb[:], wsk_d[:])
    ident_sb = const.tile([128, 128], F32R, tag="ident")
    nc.sync.dma_start(ident_sb[:], ident_d[:])
    wo1_sb = const.tile([128, 4 * 128], BF16, tag="wo1")
    nc.sync.dma_start(wo1_sb[:], wo1_d[:])
    wo2_sb = const.tile([128, 4 * 128], BF16, tag="wo2")
    nc.sync.dma_start(wo2_sb[:], wo2_d[:])

    s_sb = const.tile([128, NB], F32R, tag="scap")

    # ---- input 1x1 conv: h0 = w_in * x + b_in  (K=16 matmul broadcast),
    # chunked so block 0's h-taps can start on the first half early
    ps_h = pspool.tile([128, L], F32, tag="ps")
    h_cur = hpool.tile([128, L], BF16, tag="h")
    for (a0, a1) in ((0, 256), (256, L)):
        nc.tensor.matmul(ps_h[:, a0:a1], win_sb[:], x_sb[:, a0:a1],
                         start=True, stop=True, skip_group_check=True)
        nc.scalar.activation(h_cur[:, a0:a1], ps_h[:, a0:a1], AF.Identity,
                             bias=btiles["bin"][:, 0:1])

    # Receptive-field window: block i only needs output cols [c0[i], 512).
    c0 = [0] * NB
    need = 511
    for i in reversed(range(NB)):
        c0[i] = max(0, need & ~1)
        need = max(0, c0[i] - _dil(i))

    # ---- 32 residual blocks, factorized: conv reads h_{i-1} and fg_{i-1}.
    # h-taps (2 blocks of slack) are emitted one block EARLY so that when
    # fg_{i-1} lands only the 2 fg-tap matmuls remain on the chain.
    def _chunks(i):
        o0 = c0[i]
        mid = ((o0 + L) // 2) & ~1
        return [(o0, mid), (mid, L)] if mid - o0 >= 2 else [(o0, L)]

    def emit_h_taps(i, ps, h_tap):
        """taps of block i applied to h_{i-1}; also delta bias (consts)."""
        d = _dil(i)
        w_d1, w_d0, w_d0s = wstrip(i, 0), wstrip(i, 1), wstrip(i, 2)
        for (a0, a1) in _chunks(i):
            i0 = max(a0, d)
            mm = [(slice(a0, a1), w_d1, h_tap[:, a0:a1]),
                  (slice(i0, a1), w_d0, h_tap[:, i0 - d:a1 - d])]
            if a0 < d:
                e = min(d, a1)
                mm.append((slice(a0, e), w_d0s,
                           h_tap[:, L - d + a0:L - d + e]))
                if i > 0:
                    mm.append((slice(a0, e),
                               wdelta_sb[0:1, 128 * i:128 * (i + 1)],
                               ones_sb[0:1, 0:e - a0]))
            for k, (osl, wv, rv) in enumerate(mm):
                nc.tensor.matmul(ps[:, osl], wv, rv, start=(k == 0),
                                 stop=(i == 0 and k == len(mm) - 1),
                                 skip_group_check=True)

    def emit_fg_taps(i, ps, fg_tap):
        d = _dil(i)
        w_d1o, w_d0o, w_d0so = wstrip(i, 6), wstrip(i, 7), wstrip(i, 8)
        for (a0, a1) in _chunks(i):
            i0 = max(a0, d)
            mm = [(slice(a0, a1), w_d1o, fg_tap[:, a0:a1]),
                  (slice(i0, a1), w_d0o, fg_tap[:, i0 - d:a1 - d])]
            if a0 < d:
                e = min(d, a1)
                mm.append((slice(a0, e), w_d0so,
                           fg_tap[:, L - d + a0:L - d + e]))
            for k, (osl, wv, rv) in enumerate(mm):
                nc.tensor.matmul(ps[:, osl], wv, rv, start=False,
                                 stop=(k == len(mm) - 1),
                                 skip_group_check=True)

    ps_next = pspool.tile([128, L], F32, tag="ps", name="ps_a0")
    emit_h_taps(0, ps_next, h_cur)
    h_prev = None
    fg_prev = None
    for i in range(NB):
        d = _dil(i)
        o0 = c0[i]
        chunks = _chunks(i)
        last = i == NB - 1

        w_f, w_g, w_o = wstrip(i, 3), wstrip(i, 4), wstrip(i, 5)

        ps_a = ps_next
        if i > 0:
            emit_fg_taps(i, ps_a, fg_prev)

        if last:
            # capture relu(a)[:, -1] for the skip head (bias_hi in bias)
            nc.scalar.activation(s_sb[:, i:i + 1], ps_a[:, L - 1:L],
                                 AF.Relu, bias=btiles["bh"][:, i:i + 1])
            break

        a_sb = work.tile([128, L], BF16, tag="a")
        ps_f = pspool_f.tile([128, L], F32, tag="psf")
        ps_g = pspool_g.tile([128, L], F32, tag="psg")
        f_sb = work.tile([128, L], BF16, tag="f")
        g_sb = work.tile([128, L], BF16, tag="g")
        fg_sb = work.tile([128, L], BF16, tag="fg")
        mk_h = i < NB - 2   # h_{i+1} unused for the last two blocks
        if mk_h:
            ps_o = pspool_o.tile([128, L], F32, tag="pso")
            h_new = hpool.tile([128, L], BF16, tag="h")
        # phase 1: both relus first — keeps reluB from queueing behind
        # fgA/hnewA on the in-order DVE (head-of-line blocking)
        for k, (a0, a1) in enumerate(chunks):
            cs = slice(a0, a1)
            nc.vector.tensor_scalar(a_sb[:, cs], ps_a[:, cs],
                                    btiles["bh"][:, i:i + 1], 0.0,
                                    op0=ALU.add, op1=ALU.max)
        for k, (a0, a1) in enumerate(chunks):
            cs = slice(a0, a1)
            nc.tensor.matmul(ps_f[:, cs], w_f, a_sb[:, cs],
                             start=True, stop=True, skip_group_check=True)
            nc.tensor.matmul(ps_g[:, cs], w_g, a_sb[:, cs],
                             start=True, stop=True, skip_group_check=True)
            nc.scalar.activation(f_sb[:, cs], ps_f[:, cs], AF.Tanh,
                                 bias=btiles["bf"][:, i:i + 1])
            nc.scalar.activation(g_sb[:, cs], ps_g[:, cs], AF.Sigmoid,
                                 bias=btiles["bg"][:, i:i + 1])
            nc.vector.tensor_tensor(fg_sb[:, cs], f_sb[:, cs], g_sb[:, cs],
                                    op=ALU.mult)
            if mk_h:
                nc.tensor.matmul(ps_o[:, cs], w_o, fg_sb[:, cs],
                                 start=True, stop=True, skip_group_check=True)
        # phase 3: h updates last on DVE (off-chain; full block of slack)
        if mk_h:
            for k, (a0, a1) in enumerate(chunks):
                cs = slice(a0, a1)
                # h_new = (ps_o + b_out) + h_cur in one fused DVE op
                nc.vector.scalar_tensor_tensor(h_new[:, cs], ps_o[:, cs],
                                               btiles["bo"][:, i:i + 1],
                                               h_cur[:, cs],
                                               op0=ALU.add, op1=ALU.add)
        # capture relu(a)[:, -1] at block end (off the critical chain)
        nc.scalar.activation(s_sb[:, i:i + 1], ps_a[:, L - 1:L],
                             AF.Relu, bias=btiles["bh"][:, i:i + 1])
        # pre-emit block i+1's h-taps (they read h_i, ready since i-1)
        ps_next = pspool.tile([128, L], F32, tag="ps", name=f"ps_a{i + 1}")
        emit_h_taps(i + 1, ps_next, h_cur)
        h_prev = h_cur
        if mk_h:
            h_cur = h_new
        fg_prev = fg_sb

    # ---- skip head: PE-transpose S [128, NB] -> [NB, 128], then contract
    ps_t = pspool1.tile([NB, 128], F32R, tag="pst")
    nc.tensor.transpose(ps_t[:], s_sb[:], ident_sb[:])
    st_sb = work.tile([NB, 128], BF16, tag="st")
    nc.scalar.copy(st_sb[:], ps_t[:])

    s1 = []
    ps_sk = [pspool.tile([128, BL], F32, tag="ps", name=f"ps_sk{_m}")
             for _m in range(2)]
    for mh in range(2):
        for c in range(C):
            wv = wsk_sb[:, (c * 2 + mh) * 128:(c * 2 + mh + 1) * 128]
            rv = st_sb[:, 24 + c::32]
            nc.tensor.matmul(ps_sk[mh][:], wv, rv,
                             start=(c == 0), stop=(c == C - 1))
    for mh in range(2):
        t = work.tile([128, BL], BF16, tag=f"s1_{mh}")
        nc.scalar.activation(t[:], ps_sk[mh][:], AF.Relu,
                             bias=btiles["bsk"][:, mh:mh + 1])
        s1.append(t)
    s2 = []
    for mh in range(2):
        ps1 = pspool.tile([128, BL], F32, tag="ps")
        for kh in range(2):
            wv = wo1_sb[:, (kh * 2 + mh) * 128:(kh * 2 + mh + 1) * 128]
            nc.tensor.matmul(ps1[:], wv, s1[kh][:],
                             start=(kh == 0), stop=(kh == 1))
        t = work.tile([128, BL], BF16, tag=f"s2_{mh}")
        nc.scalar.activation(t[:], ps1[:], AF.Relu,
                             bias=btiles["bo1"][:, mh:mh + 1])
        s2.append(t)
    y_all = work.tile([128, 2 * BL], F32R, tag="yall")
    for mh in range(2):
        ps2 = pspool.tile([128, BL], F32, tag="ps")
        for kh in range(2):
            wv = wo2_sb[:, (kh * 2 + mh) * 128:(kh * 2 + mh + 1) * 128]
            nc.tensor.matmul(ps2[:], wv, s2[kh][:],
                             start=(kh == 0), stop=(kh == 1))
        nc.scalar.activation(y_all[:, BL * mh:BL * (mh + 1)], ps2[:],
                             AF.Identity, bias=btiles["bo2"][:, mh:mh + 1])

    # transpose y so the output DMA writes contiguous 512B lines
    ps_y = pspool.tile([2 * BL, 128], F32R, tag="ps", name="ps_y")
    nc.tensor.transpose(ps_y[:], y_all[:], ident_sb[:])
    y_t = work.tile([2 * BL, 128], F32, tag="yt")
    nc.scalar.copy(y_t[:], ps_y[:])
    out_view = out_d.rearrange("b (mh o) -> mh b o", mh=2)
    for mh in range(2):
        nc.sync.dma_start(out_view[mh], y_t[BL * mh:BL * (mh + 1), :])

    ctx.close()


# ------------------------------------------------- wait-split fixup


def _split_excess_waits(nc, cap=2):
    counter = [0]
    for fn in nc.m.functions:
        for blk in fn.blocks:
            insts = blk.instructions
            out = []
            changed = False
            for inst in insts:
                cap = 1
                si = inst.sync_info
                waits = list(si.on_wait) if si is not None else []
                if len(waits) > cap:
                    changed = True
                    extra, keep = waits[:-cap], waits[-cap:]
                    for j in range(0, len(extra), cap):
                        grp = extra[j:j + cap]
                        nop = mybir.InstNoOp(
                            name=f"wait-split-{counter[0]}", ins=[], outs=[])
                        counter[0] += 1
                        nop.engine = inst.engine
                        nop.sync_info = mybir.SyncInfo(on_wait=grp,
                                                       on_update=[])
                        out.append(nop)
                    inst.sync_info = mybir.SyncInfo(
                        on_wait=keep, on_update=list(si.on_update))
                out.append(inst)
            if changed:
                blk.instructions = out


# ------------------------------------------------- host-side packing


def _pack_params(p):
    """Build the device-layout parameter arrays (replicated per core)."""
    f32 = np.float32
    bf16 = ml_dtypes.bfloat16
    w_in = np.asarray(p["w_in"], f32)       # (8, 1)
    b_in = np.asarray(p["b_in"], f32)       # (8,)
    w_dil = np.asarray(p["w_dil"], f32)     # (NB, 8, 8, 2)
    b_dil = np.asarray(p["b_dil"], f32)     # (NB, 8)
    w_skip = np.asarray(p["w_skip"], f32)   # (NB, SK, 8)
    b_skip = np.asarray(p["b_skip"], f32)   # (NB, SK)
    w_f = np.asarray(p["w_f"], f32)
    b_f = np.asarray(p["b_f"], f32)
    w_g = np.asarray(p["w_g"], f32)
    b_g = np.asarray(p["b_g"], f32)
    w_out = np.asarray(p["w_out"], f32)
    b_out = np.asarray(p["b_out"], f32)
    w_o1 = np.asarray(p["w_o1"], f32)       # (SK, SK)
    b_o1 = np.asarray(p["b_o1"], f32)
    w_o2 = np.asarray(p["w_o2"], f32)
    b_o2 = np.asarray(p["b_o2"], f32)

    cvec = np.arange(128) % C               # channel per partition

    def strip_pack(mat_per_block, shifted=False):
        """mat_per_block: (NB, 8out, 8in) -> [NB, 128, 128] block-diag lhsT."""
        out = np.zeros((NB, 128, 128), f32)
        for i in range(NB):
            for j in range(16):
                if shifted:
                    if j % 4 == 0:
                        continue
                    krow = 8 * (j - 1)
                else:
                    krow = 8 * j
                mcol = 8 * j
                out[i, krow:krow + 8, mcol:mcol + 8] = mat_per_block[i].T
        return out

    # factorized fg-tap weights: wd{1,0}_i @ wo_{i-1}  (zero for block 0)
    wo_prev = np.concatenate([np.zeros((1, C, C), f32), w_out[:-1]], axis=0)
    wd1o = np.einsum('iok,ikc->ioc', w_dil[:, :, :, 1], wo_prev)
    wd0o = np.einsum('iok,ikc->ioc', w_dil[:, :, :, 0], wo_prev)
    bo_prev = np.concatenate([np.zeros((1, C), f32), b_out[:-1]], axis=0)

    wall_h = np.concatenate([
        strip_pack(w_dil[:, :, :, 1]),
        strip_pack(w_dil[:, :, :, 0]),
        strip_pack(w_dil[:, :, :, 0], shifted=True),
        strip_pack(w_f),
        strip_pack(w_g),
        strip_pack(w_out),
        strip_pack(wd1o),
        strip_pack(wd0o),
        strip_pack(wd0o, shifted=True),
    ], axis=2).astype(bf16)

    # bias_hi[:, i] = bd_i + ((wd1_i + wd0_i) @ bo_{i-1})  (per channel)
    bh8 = b_dil + np.einsum('iok,ik->io', w_dil[:, :, :, 1] + w_dil[:, :, :, 0],
                            bo_prev)
    bh_h = bh8.T[cvec, :].astype(f32)       # [128, NB]

    # boundary delta row: cols t<d on tc=0 groups lack the wd0 shift term
    dvec8 = -np.einsum('iok,ik->io', w_dil[:, :, :, 0], bo_prev)  # [NB, 8]
    wdelta_h = np.zeros((1, NB * 128), f32)
    for i in range(NB):
        for m in range(128):
            if (m % 32) < 8:
                wdelta_h[0, 128 * i + m] = dvec8[i, m % 8]
    ones_h = np.ones((1, 128), f32)

    win_h = np.zeros((BL * TCH, 128), f32)
    for j in range(BL * TCH):
        win_h[j, 8 * j:8 * j + 8] = w_in[:, 0]

    bin_h = b_in[cvec][:, None].astype(f32)
    bf_h = b_f.T[cvec, :]
    bg_h = b_g.T[cvec, :]
    bo_h = b_out.T[cvec, :]

    wsk_h = np.zeros((NB, 16 * 128), f32)
    for c in range(C):
        for mh in range(2):
            wsk_h[:, (c * 2 + mh) * 128:(c * 2 + mh + 1) * 128] = \
                w_skip[:, 128 * mh:128 * (mh + 1), c]

    wo1_h4 = np.zeros((2, 2, 128, 128), f32)
    wo2_h4 = np.zeros((2, 2, 128, 128), f32)
    for kh in range(2):
        for mh in range(2):
            wo1_h4[kh, mh] = w_o1[128 * mh:128 * (mh + 1),
                                  128 * kh:128 * (kh + 1)].T
            wo2_h4[kh, mh] = w_o2[128 * mh:128 * (mh + 1),
                                  128 * kh:128 * (kh + 1)].T

    bsk_h = np.stack([b_skip.sum(0)[:128], b_skip.sum(0)[128:]], axis=1)
    bo1_h = np.stack([b_o1[:128], b_o1[128:]], axis=1)
    bo2_h = np.stack([b_o2[:128], b_o2[128:]], axis=1)

    def flat4(w):
        return np.concatenate([w[kh, mh] for kh in range(2) for mh in range(2)],
                              axis=1)

    wo1_h = flat4(wo1_h4).astype(bf16)
    wo2_h = flat4(wo2_h4).astype(bf16)
    ident_h = np.eye(128, dtype=f32)

    return dict(win=win_h, wall=wall_h, wdelta=wdelta_h.astype(bf16),
                ones=ones_h.astype(bf16), bin=bin_h, bh=bh_h, bf=bf_h,
                bg=bg_h, bo=bo_h, wsk=wsk_h.astype(bf16), ident=ident_h,
                bsk=bsk_h, wo1=wo1_h, bo1=bo1_h, wo2=wo2_h, bo2=bo2_h)


_NC_CACHE = {}


def get_nc(fixup=True):
    key = "nc" if fixup else "nc_nofix"
    if key not in _NC_CACHE:
        _NC_CACHE[key] = _build_nc(fixup=fixup)
    return _NC_CACHE[key]


def make_in_maps(**inputs):
    x = np.asarray(inputs["x"], np.float32)
    params = _pack_params(inputs)
    in_maps = []
    for k in range(N_CORES):
        shard = x[BL * k:BL * (k + 1)]                       # (4, 2048)
        x_l = shard.reshape(BL, TCH, L).reshape(BL * TCH, L).copy()
        m = {"x": x_l}
        m.update(params)
        in_maps.append(m)
    return in_maps


def kernel(**inputs):
    nc = get_nc()
    in_maps = make_in_maps(**inputs)
    res = run_bass_kernel_spmd(nc, in_maps, list(range(N_CORES)))
    outs = [res.results[k]["out"] for k in range(N_CORES)]
    return np.concatenate(outs, axis=0).astype(np.float32)


# revision 30
# speedup vs baseline: 1.0125x; 1.0125x over previous
"""WaveNet-like dense CNN on 8 TRN2 NeuronCores — batch data parallel.

v4: factorized residual conv. Block i's conv-psum is computed as
  taps_i(h_{i-1}) + (wd@wo)-taps_i(fg_{i-1}) + bias_hi_i [+ boundary delta]
so the cross-block critical chain is fg_{i-1} -> tapsfg -> relu -> gate
mms -> tanh/sig -> fg_i, skipping the wo-matmul and h-update stages.
h itself updates off-chain with a full block of slack. bias_hi folds
bd_i + (wd1_i+wd0_i)@bo_{i-1} into the relu bias; a tiny K=1 matmul
fixes the zero-pad boundary columns on tc=0 partition groups.
"""

import sys

sys.path.insert(0, "/opt/trn_rl_repo")

import numpy as np
import ml_dtypes

import concourse.bass as bass
import concourse.tile as tile
from concourse import mybir
from concourse.bass_utils import run_bass_kernel_spmd

F32 = mybir.dt.float32
F32R = mybir.dt.float32r
BF16 = mybir.dt.bfloat16
AF = mybir.ActivationFunctionType
ALU = mybir.AluOpType

SEQ_LEN = 2048
C = 8
SK = 256
NB = 32
B = 32
N_CORES = 8
BL = B // N_CORES      # 4 samples per core
TCH = 4                # time chunks per sample
L = SEQ_LEN // TCH     # 512 free elems
NW = 9                 # packed weight strips per block


def _dil(i):
    return 2 ** (i % 8)


# ---------------------------------------------------------------- build


def _build_nc(fixup=True):
    nc = bass.Bass("TRN2", target_bir_lowering=False, debug=False,
                   num_devices=N_CORES)

    def din(name, shape, dt=F32R):
        return nc.dram_tensor(name, shape, dt, kind="ExternalInput").ap()

    x_d = din("x", [BL * TCH, L])
    win_d = din("win", [BL * TCH, 128])
    # per-block strips packed contiguously (order: wd1, wd0, wd0s, wf, wg,
    # wo, wd1o, wd0o, wd0so); one DMA per block
    wall_d = din("wall", [NB, 128, NW * 128], BF16)
    wdelta_d = din("wdelta", [1, NB * 128], BF16)
    ones_d = din("ones", [1, 128], BF16)
    bin_d = din("bin", [128, 1], F32)
    bh_d = din("bh", [128, NB], F32)
    bf_d = din("bf", [128, NB], F32)
    bg_d = din("bg", [128, NB], F32)
    bo_d = din("bo", [128, NB], F32)
    wsk_d = din("wsk", [32, 16 * 128], BF16)
    ident_d = din("ident", [128, 128])
    bsk_d = din("bsk", [128, 2], F32)
    wo1_d = din("wo1", [128, 4 * 128], BF16)
    bo1_d = din("bo1", [128, 2], F32)
    wo2_d = din("wo2", [128, 4 * 128], BF16)
    bo2_d = din("bo2", [128, 2], F32)

    out_d = nc.dram_tensor("out", [BL, SK], F32, kind="ExternalOutput").ap()

    with tile.TileContext(nc) as tc:
        _emit(nc, tc, x_d, win_d, wall_d, wdelta_d, ones_d,
              bin_d, bh_d, bf_d, bg_d, bo_d, wsk_d, ident_d, bsk_d,
              wo1_d, bo1_d, wo2_d, bo2_d, out_d)

    if fixup:
        _split_excess_waits(nc)
    return nc


def _emit(nc, tc, x_d, win_d, wall_d, wdelta_d, ones_d,
          bin_d, bh_d, bf_d, bg_d, bo_d, wsk_d, ident_d, bsk_d,
          wo1_d, bo1_d, wo2_d, bo2_d, out_d):
    from contextlib import ExitStack
    ctx = ExitStack()
    const = ctx.enter_context(tc.tile_pool(name="const", bufs=1))
    work = ctx.enter_context(tc.tile_pool(name="work", bufs=2))
    hpool = ctx.enter_context(tc.tile_pool(name="h", bufs=3))
    pspool = ctx.enter_context(tc.tile_pool(name="ps", bufs=3, space="PSUM"))
    pspool_f = ctx.enter_context(tc.tile_pool(name="psf", bufs=1, space="PSUM"))
    pspool_g = ctx.enter_context(tc.tile_pool(name="psg", bufs=1, space="PSUM"))
    pspool_o = ctx.enter_context(tc.tile_pool(name="pso", bufs=2, space="PSUM"))
    pspool1 = ctx.enter_context(tc.tile_pool(name="ps1", bufs=1, space="PSUM"))

    # ---- persistent loads (x/win/bias + first blocks' weights first so
    # compute starts early; the rest streams behind)
    x_sb = const.tile([BL * TCH, L], F32R, tag="x")
    nc.sync.dma_start(x_sb[:], x_d[:])
    win_sb = const.tile([BL * TCH, 128], F32R, tag="win")
    nc.sync.dma_start(win_sb[:], win_d[:])

    btiles = {}
    wall_sb = const.tile([128, NB * NW * 128], BF16, tag="wall")

    def dma_wall(i):
        nc.sync.dma_start(
            wall_sb[:, NW * 128 * i:NW * 128 * (i + 1)], wall_d[i])

    bspecs = (("bin", bin_d, 1), ("bh", bh_d, NB), ("bf", bf_d, NB),
              ("bg", bg_d, NB), ("bo", bo_d, NB), ("bsk", bsk_d, 2),
              ("bo1", bo1_d, 2), ("bo2", bo2_d, 2))
    for nm, d, w in bspecs:
        btiles[nm] = const.tile([128, w], F32, tag=nm, name=f"b_{nm}")
    # issue the startup-critical transfers on idle engines' queues so they
    # run in parallel instead of serializing on the sync queue
    nc.scalar.dma_start(btiles["bin"][:], bin_d[:])
    nc.gpsimd.dma_start(
        wall_sb[:, 0:NW * 128], wall_d[0])
    nc.scalar.dma_start(btiles["bh"][:], bh_d[:])
    # warm the Act table (sigmoid_and_others covers identity/relu/tanh/
    # sigmoid): the 1.3us load overlaps the initial DMA wait instead of
    # blocking the first real activation. Emitted AFTER the scalar-queue
    # DMA issues so it does not delay them.
    warm = const.tile([1, 2], F32, tag="warm")
    nc.vector.memset(warm[:], 0.0)
    nc.scalar.activation(warm[:], warm[:], AF.Sigmoid)
    wdelta_sb = const.tile([1, NB * 128], BF16, tag="wdelta")
    nc.sync.dma_start(wdelta_sb[:], wdelta_d[:])
    ones_sb = const.tile([1, 128], BF16, tag="ones")
    nc.sync.dma_start(ones_sb[:], ones_d[:])
    nc.gpsimd.dma_start(
        wall_sb[:, NW * 128:2 * NW * 128], wall_d[1])
    for nm, d, w in bspecs:
        if nm not in ("bin", "bh"):
            nc.sync.dma_start(btiles[nm][:], d[:])
    for i in range(2, NB):
        dma_wall(i)

    def wstrip(i, j):
        o = NW * 128 * i + 128 * j
        return wall_sb[:, o:o + 128]

    # tail weights last (only needed at the very end)
    wsk_sb = const.tile([32, 16 * 128], BF16, tag="wsk")
    nc.sync.dma_start(wsk_sb[:], wsk_d[:])
    ident_sb = const.tile([128, 128], F32R, tag="ident")
    nc.sync.dma_start(ident_sb[:], ident_d[:])
    wo1_sb = const.tile([128, 4 * 128], BF16, tag="wo1")
    nc.sync.dma_start(wo1_sb[:], wo1_d[:])
    wo2_sb = const.tile([128, 4 * 128], BF16, tag="wo2")
    nc.sync.dma_start(wo2_sb[:], wo2_d[:])

    s_sb = const.tile([128, NB], F32R, tag="scap")

    # ---- input 1x1 conv: h0 = w_in * x + b_in  (K=16 matmul broadcast)
    ps_h = pspool.tile([128, L], F32, tag="ps")
    nc.tensor.matmul(ps_h[:], win_sb[:], x_sb[:], start=True, stop=True)
    h_cur = hpool.tile([128, L], BF16, tag="h")
    nc.scalar.activation(h_cur[:], ps_h[:], AF.Identity,
                         bias=btiles["bin"][:, 0:1])

    # Receptive-field window: block i only needs output cols [c0[i], 512).
    c0 = [0] * NB
    need = 511
    for i in reversed(range(NB)):
        c0[i] = max(0, need & ~1)
        need = max(0, c0[i] - _dil(i))

    # ---- 32 residual blocks, factorized: conv reads h_{i-1} and fg_{i-1}.
    # h-taps (2 blocks of slack) are emitted one block EARLY so that when
    # fg_{i-1} lands only the 2 fg-tap matmuls remain on the chain.
    def _chunks(i):
        o0 = c0[i]
        mid = ((o0 + L) // 2) & ~1
        return [(o0, mid), (mid, L)] if mid - o0 >= 2 else [(o0, L)]

    def emit_h_taps(i, ps, h_tap):
        """taps of block i applied to h_{i-1}; also delta bias (consts)."""
        d = _dil(i)
        w_d1, w_d0, w_d0s = wstrip(i, 0), wstrip(i, 1), wstrip(i, 2)
        for (a0, a1) in _chunks(i):
            i0 = max(a0, d)
            mm = [(slice(a0, a1), w_d1, h_tap[:, a0:a1]),
                  (slice(i0, a1), w_d0, h_tap[:, i0 - d:a1 - d])]
            if a0 < d:
                e = min(d, a1)
                mm.append((slice(a0, e), w_d0s,
                           h_tap[:, L - d + a0:L - d + e]))
                if i > 0:
                    mm.append((slice(a0, e),
                               wdelta_sb[0:1, 128 * i:128 * (i + 1)],
                               ones_sb[0:1, 0:e - a0]))
            for k, (osl, wv, rv) in enumerate(mm):
                nc.tensor.matmul(ps[:, osl], wv, rv, start=(k == 0),
                                 stop=(i == 0 and k == len(mm) - 1),
                                 skip_group_check=True)

    def emit_fg_taps(i, ps, fg_tap):
        d = _dil(i)
        w_d1o, w_d0o, w_d0so = wstrip(i, 6), wstrip(i, 7), wstrip(i, 8)
        for (a0, a1) in _chunks(i):
            i0 = max(a0, d)
            mm = [(slice(a0, a1), w_d1o, fg_tap[:, a0:a1]),
                  (slice(i0, a1), w_d0o, fg_tap[:, i0 - d:a1 - d])]
            if a0 < d:
                e = min(d, a1)
                mm.append((slice(a0, e), w_d0so,
                           fg_tap[:, L - d + a0:L - d + e]))
            for k, (osl, wv, rv) in enumerate(mm):
                nc.tensor.matmul(ps[:, osl], wv, rv, start=False,
                                 stop=(k == len(mm) - 1),
                                 skip_group_check=True)

    ps_next = pspool.tile([128, L], F32, tag="ps", name="ps_a0")
    emit_h_taps(0, ps_next, h_cur)
    h_prev = None
    fg_prev = None
    for i in range(NB):
        d = _dil(i)
        o0 = c0[i]
        chunks = _chunks(i)
        last = i == NB - 1

        w_f, w_g, w_o = wstrip(i, 3), wstrip(i, 4), wstrip(i, 5)

        ps_a = ps_next
        if i > 0:
            emit_fg_taps(i, ps_a, fg_prev)

        if last:
            # capture relu(a)[:, -1] for the skip head (bias_hi in bias)
            nc.scalar.activation(s_sb[:, i:i + 1], ps_a[:, L - 1:L],
                                 AF.Relu, bias=btiles["bh"][:, i:i + 1])
            break

        a_sb = work.tile([128, L], BF16, tag="a")
        ps_f = pspool_f.tile([128, L], F32, tag="psf")
        ps_g = pspool_g.tile([128, L], F32, tag="psg")
        f_sb = work.tile([128, L], BF16, tag="f")
        g_sb = work.tile([128, L], BF16, tag="g")
        fg_sb = work.tile([128, L], BF16, tag="fg")
        mk_h = i < NB - 2   # h_{i+1} unused for the last two blocks
        if mk_h:
            ps_o = pspool_o.tile([128, L], F32, tag="pso")
            h_new = hpool.tile([128, L], BF16, tag="h")
        # phase 1: both relus first — keeps reluB from queueing behind
        # fgA/hnewA on the in-order DVE (head-of-line blocking)
        for k, (a0, a1) in enumerate(chunks):
            cs = slice(a0, a1)
            nc.vector.tensor_scalar(a_sb[:, cs], ps_a[:, cs],
                                    btiles["bh"][:, i:i + 1], 0.0,
                                    op0=ALU.add, op1=ALU.max)
        for k, (a0, a1) in enumerate(chunks):
            cs = slice(a0, a1)
            nc.tensor.matmul(ps_f[:, cs], w_f, a_sb[:, cs],
                             start=True, stop=True, skip_group_check=True)
            nc.tensor.matmul(ps_g[:, cs], w_g, a_sb[:, cs],
                             start=True, stop=True, skip_group_check=True)
            nc.scalar.activation(f_sb[:, cs], ps_f[:, cs], AF.Tanh,
                                 bias=btiles["bf"][:, i:i + 1])
            nc.scalar.activation(g_sb[:, cs], ps_g[:, cs], AF.Sigmoid,
                                 bias=btiles["bg"][:, i:i + 1])
            nc.vector.tensor_tensor(fg_sb[:, cs], f_sb[:, cs], g_sb[:, cs],
                                    op=ALU.mult)
            if mk_h:
                nc.tensor.matmul(ps_o[:, cs], w_o, fg_sb[:, cs],
                                 start=True, stop=True, skip_group_check=True)
        # phase 3: h updates last on DVE (off-chain; full block of slack)
        if mk_h:
            for k, (a0, a1) in enumerate(chunks):
                cs = slice(a0, a1)
                # h_new = (ps_o + b_out) + h_cur in one fused DVE op
                nc.vector.scalar_tensor_tensor(h_new[:, cs], ps_o[:, cs],
                                               btiles["bo"][:, i:i + 1],
                                               h_cur[:, cs],
                                               op0=ALU.add, op1=ALU.add)
        # capture a[:, -1] (== relu(ps_a+bias) at col 511) for the skip
        # head via the idle GPSIMD engine — keeps Act free for the gates
        nc.gpsimd.tensor_copy(s_sb[:, i:i + 1], a_sb[:, L - 1:L])
        # pre-emit block i+1's h-taps (they read h_i, ready since i-1)
        ps_next = pspool.tile([128, L], F32, tag="ps", name=f"ps_a{i + 1}")
        emit_h_taps(i + 1, ps_next, h_cur)
        h_prev = h_cur
        if mk_h:
            h_cur = h_new
        fg_prev = fg_sb

    # ---- skip head: PE-transpose S [128, NB] -> [NB, 128], then contract
    ps_t = pspool1.tile([NB, 128], F32R, tag="pst")
    nc.tensor.transpose(ps_t[:], s_sb[:], ident_sb[:])
    st_sb = work.tile([NB, 128], BF16, tag="st")
    nc.scalar.copy(st_sb[:], ps_t[:])

    s1 = []
    ps_sk = [pspool.tile([128, BL], F32, tag="ps", name=f"ps_sk{_m}")
             for _m in range(2)]
    for mh in range(2):
        for c in range(C):
            wv = wsk_sb[:, (c * 2 + mh) * 128:(c * 2 + mh + 1) * 128]
            rv = st_sb[:, 24 + c::32]
            nc.tensor.matmul(ps_sk[mh][:], wv, rv,
                             start=(c == 0), stop=(c == C - 1))
    for mh in range(2):
        t = work.tile([128, BL], BF16, tag=f"s1_{mh}")
        nc.scalar.activation(t[:], ps_sk[mh][:], AF.Relu,
                             bias=btiles["bsk"][:, mh:mh + 1])
        s1.append(t)
    s2 = []
    for mh in range(2):
        ps1 = pspool.tile([128, BL], F32, tag="ps")
        for kh in range(2):
            wv = wo1_sb[:, (kh * 2 + mh) * 128:(kh * 2 + mh + 1) * 128]
            nc.tensor.matmul(ps1[:], wv, s1[kh][:],
                             start=(kh == 0), stop=(kh == 1))
        t = work.tile([128, BL], BF16, tag=f"s2_{mh}")
        nc.scalar.activation(t[:], ps1[:], AF.Relu,
                             bias=btiles["bo1"][:, mh:mh + 1])
        s2.append(t)
    y_all = work.tile([128, 2 * BL], F32R, tag="yall")
    for mh in range(2):
        ps2 = pspool.tile([128, BL], F32, tag="ps")
        for kh in range(2):
            wv = wo2_sb[:, (kh * 2 + mh) * 128:(kh * 2 + mh + 1) * 128]
            nc.tensor.matmul(ps2[:], wv, s2[kh][:],
                             start=(kh == 0), stop=(kh == 1))
        nc.scalar.activation(y_all[:, BL * mh:BL * (mh + 1)], ps2[:],
                             AF.Identity, bias=btiles["bo2"][:, mh:mh + 1])

    # transpose y so the output DMA writes contiguous 512B lines
    ps_y = pspool.tile([2 * BL, 128], F32R, tag="ps", name="ps_y")
    nc.tensor.transpose(ps_y[:], y_all[:], ident_sb[:])
    y_t = work.tile([2 * BL, 128], F32, tag="yt")
    nc.scalar.copy(y_t[:], ps_y[:])
    out_view = out_d.rearrange("b (mh o) -> mh b o", mh=2)
    for mh in range(2):
        nc.sync.dma_start(out_view[mh], y_t[BL * mh:BL * (mh + 1), :])

    ctx.close()


# ------------------------------------------------- wait-split fixup


def _split_excess_waits(nc, cap=2):
    counter = [0]
    for fn in nc.m.functions:
        for blk in fn.blocks:
            insts = blk.instructions
            out = []
            changed = False
            for inst in insts:
                cap = 1
                si = inst.sync_info
                waits = list(si.on_wait) if si is not None else []
                if len(waits) > cap:
                    changed = True
                    extra, keep = waits[:-cap], waits[-cap:]
                    for j in range(0, len(extra), cap):
                        grp = extra[j:j + cap]
                        nop = mybir.InstNoOp(
                            name=f"wait-split-{counter[0]}", ins=[], outs=[])
                        counter[0] += 1
                        nop.engine = inst.engine
                        nop.sync_info = mybir.SyncInfo(on_wait=grp,
                                                       on_update=[])
                        out.append(nop)
                    inst.sync_info = mybir.SyncInfo(
                        on_wait=keep, on_update=list(si.on_update))
                out.append(inst)
            if changed:
                blk.instructions = out


# ------------------------------------------------- host-side packing


def _pack_params(p):
    """Build the device-layout parameter arrays (replicated per core)."""
    f32 = np.float32
    bf16 = ml_dtypes.bfloat16
    w_in = np.asarray(p["w_in"], f32)       # (8, 1)
    b_in = np.asarray(p["b_in"], f32)       # (8,)
    w_dil = np.asarray(p["w_dil"], f32)     # (NB, 8, 8, 2)
    b_dil = np.asarray(p["b_dil"], f32)     # (NB, 8)
    w_skip = np.asarray(p["w_skip"], f32)   # (NB, SK, 8)
    b_skip = np.asarray(p["b_skip"], f32)   # (NB, SK)
    w_f = np.asarray(p["w_f"], f32)
    b_f = np.asarray(p["b_f"], f32)
    w_g = np.asarray(p["w_g"], f32)
    b_g = np.asarray(p["b_g"], f32)
    w_out = np.asarray(p["w_out"], f32)
    b_out = np.asarray(p["b_out"], f32)
    w_o1 = np.asarray(p["w_o1"], f32)       # (SK, SK)
    b_o1 = np.asarray(p["b_o1"], f32)
    w_o2 = np.asarray(p["w_o2"], f32)
    b_o2 = np.asarray(p["b_o2"], f32)

    cvec = np.arange(128) % C               # channel per partition

    def strip_pack(mat_per_block, shifted=False):
        """mat_per_block: (NB, 8out, 8in) -> [NB, 128, 128] block-diag lhsT."""
        out = np.zeros((NB, 128, 128), f32)
        for i in range(NB):
            for j in range(16):
                if shifted:
                    if j % 4 == 0:
                        continue
                    krow = 8 * (j - 1)
                else:
                    krow = 8 * j
                mcol = 8 * j
                out[i, krow:krow + 8, mcol:mcol + 8] = mat_per_block[i].T
        return out

    # factorized fg-tap weights: wd{1,0}_i @ wo_{i-1}  (zero for block 0)
    wo_prev = np.concatenate([np.zeros((1, C, C), f32), w_out[:-1]], axis=0)
    wd1o = np.einsum('iok,ikc->ioc', w_dil[:, :, :, 1], wo_prev)
    wd0o = np.einsum('iok,ikc->ioc', w_dil[:, :, :, 0], wo_prev)
    bo_prev = np.concatenate([np.zeros((1, C), f32), b_out[:-1]], axis=0)

    wall_h = np.concatenate([
        strip_pack(w_dil[:, :, :, 1]),
        strip_pack(w_dil[:, :, :, 0]),
        strip_pack(w_dil[:, :, :, 0], shifted=True),
        strip_pack(w_f),
        strip_pack(w_g),
        strip_pack(w_out),
        strip_pack(wd1o),
        strip_pack(wd0o),
        strip_pack(wd0o, shifted=True),
    ], axis=2).astype(bf16)

    # bias_hi[:, i] = bd_i + ((wd1_i + wd0_i) @ bo_{i-1})  (per channel)
    bh8 = b_dil + np.einsum('iok,ik->io', w_dil[:, :, :, 1] + w_dil[:, :, :, 0],
                            bo_prev)
    bh_h = bh8.T[cvec, :].astype(f32)       # [128, NB]

    # boundary delta row: cols t<d on tc=0 groups lack the wd0 shift term
    dvec8 = -np.einsum('iok,ik->io', w_dil[:, :, :, 0], bo_prev)  # [NB, 8]
    wdelta_h = np.zeros((1, NB * 128), f32)
    for i in range(NB):
        for m in range(128):
            if (m % 32) < 8:
                wdelta_h[0, 128 * i + m] = dvec8[i, m % 8]
    ones_h = np.ones((1, 128), f32)

    win_h = np.zeros((BL * TCH, 128), f32)
    for j in range(BL * TCH):
        win_h[j, 8 * j:8 * j + 8] = w_in[:, 0]

    bin_h = b_in[cvec][:, None].astype(f32)
    bf_h = b_f.T[cvec, :]
    bg_h = b_g.T[cvec, :]
    bo_h = b_out.T[cvec, :]

    wsk_h = np.zeros((NB, 16 * 128), f32)
    for c in range(C):
        for mh in range(2):
            wsk_h[:, (c * 2 + mh) * 128:(c * 2 + mh + 1) * 128] = \
                w_skip[:, 128 * mh:128 * (mh + 1), c]

    wo1_h4 = np.zeros((2, 2, 128, 128), f32)
    wo2_h4 = np.zeros((2, 2, 128, 128), f32)
    for kh in range(2):
        for mh in range(2):
            wo1_h4[kh, mh] = w_o1[128 * mh:128 * (mh + 1),
                                  128 * kh:128 * (kh + 1)].T
            wo2_h4[kh, mh] = w_o2[128 * mh:128 * (mh + 1),
                                  128 * kh:128 * (kh + 1)].T

    bsk_h = np.stack([b_skip.sum(0)[:128], b_skip.sum(0)[128:]], axis=1)
    bo1_h = np.stack([b_o1[:128], b_o1[128:]], axis=1)
    bo2_h = np.stack([b_o2[:128], b_o2[128:]], axis=1)

    def flat4(w):
        return np.concatenate([w[kh, mh] for kh in range(2) for mh in range(2)],
                              axis=1)

    wo1_h = flat4(wo1_h4).astype(bf16)
    wo2_h = flat4(wo2_h4).astype(bf16)
    ident_h = np.eye(128, dtype=f32)

    return dict(win=win_h, wall=wall_h, wdelta=wdelta_h.astype(bf16),
                ones=ones_h.astype(bf16), bin=bin_h, bh=bh_h, bf=bf_h,
                bg=bg_h, bo=bo_h, wsk=wsk_h.astype(bf16), ident=ident_h,
                bsk=bsk_h, wo1=wo1_h, bo1=bo1_h, wo2=wo2_h, bo2=bo2_h)


_NC_CACHE = {}


def get_nc(fixup=True):
    key = "nc" if fixup else "nc_nofix"
    if key not in _NC_CACHE:
        _NC_CACHE[key] = _build_nc(fixup=fixup)
    return _NC_CACHE[key]


def make_in_maps(**inputs):
    x = np.asarray(inputs["x"], np.float32)
    params = _pack_params(inputs)
    in_maps = []
    for k in range(N_CORES):
        shard = x[BL * k:BL * (k + 1)]                       # (4, 2048)
        x_l = shard.reshape(BL, TCH, L).reshape(BL * TCH, L).copy()
        m = {"x": x_l}
        m.update(params)
        in_maps.append(m)
    return in_maps


def kernel(**inputs):
    nc = get_nc()
    in_maps = make_in_maps(**inputs)
    res = run_bass_kernel_spmd(nc, in_maps, list(range(N_CORES)))
    outs = [res.results[k]["out"] for k in range(N_CORES)]
    return np.concatenate(outs, axis=0).astype(np.float32)
